# revision 8
# baseline (speedup 1.0000x reference)
"""Cost-volume kernel for Trainium2 (Bass/Tile), SPMD over 8 NeuronCores.

out[n, c, d, h, x] = l[n, c, h, x] - r[n, c, h, x - d]  for x >= d, else 1.0
shapes: l, r = (2, 32, 128, 256) f32 -> out = (2, 32, 48, 128, 256) f32

Sharding: the 64 (n, c) pairs split 8 ways -> G=8 channels per core; no
cross-core communication.

The kernel is output-write bound: trace analysis showed all 16 SDMA engines
~100% busy (2 KB descriptors, ~22.4 GB/s per engine, ~360 GB/s aggregate),
so the levers are all byte-count:
  1. fp16 device pipeline (inputs pre-cast on host, DVE subtract fp16,
     output DMA fp16) — halves traffic vs f32; ~5e-4 scale-rel error
     against the 2e-2 gate. Host upcasts on gather.
  2. The constant x < d triangle (9.2% of output) is never written: per
     disparity the DVE writes a packed [128, 8*(W-d)] tile; the DMA lands
     each partition's payload as two DRAM rows of 1024-4d elements (+8
     pad), keeping ~2 KB single-fragment descriptors. The host scatters
     the valid region into the final array and fills the triangle with 1.

Per-core layout: SBUF partition p = (g, h_hi), per-partition free dims
(h_lo=8, w). DRAM payload rows are padded by 8 elements (the 16 B gap
defeats descriptor coalescing), so each row is one descriptor and the
outer DRAM AP dim (256) sprays descriptors across all 16 SDMA engines.
Measured on HW: an outer dim of 8 engages only 8 engines (halves DMA
bandwidth); 8 KB descriptors run at ~0.7x the per-engine rate of 2 KB
ones. One DVE subtract per disparity covers all channels; output DMAs
alternate between the two HWDGE rings.
"""

import numpy as np

import concourse.bacc as bacc
import concourse.mybir as mybir
import concourse.tile as tile
from concourse.bass_utils import run_bass_kernel_spmd

MAX_DISP = 48
N, C, H, W = 2, 32, 128, 256
NCORES = 8
G = (N * C) // NCORES  # 8 (n, c) channels per core
HL = 8  # h_lo rows per partition; 128 partitions = G * (H // HL)
IROWS = H * W // 1024  # 32 input payload rows of 1024 fp16 per g
IPAD = 1032
# output: per disparity, 256 DRAM rows (2 per partition) of 1024-4d payload
# elements (= 4 h-rows of W-d) padded by 8
OROW = [1032 - 4 * d for d in range(MAX_DISP)]
OPAY = [1024 - 4 * d for d in range(MAX_DISP)]
OFF = np.cumsum([0] + [256 * r for r in OROW]).tolist()
OSIZE = OFF[-1]

_CACHE = {}


def build_bass():
    if "nc" in _CACHE:
        return _CACHE["nc"]
    nc = bacc.Bacc("TRN2", target_bir_lowering=False, debug=False)
    l = nc.dram_tensor("l", (G, IROWS, IPAD), mybir.dt.bfloat16, kind="ExternalInput")
    r = nc.dram_tensor("r", (G, IROWS, IPAD), mybir.dt.bfloat16, kind="ExternalInput")
    out = nc.dram_tensor("out", (OSIZE,), mybir.dt.float16, kind="ExternalOutput")

    with tile.TileContext(nc) as tc:
        with tc.tile_pool(name="inp", bufs=1) as inpool, tc.tile_pool(
            name="outp", bufs=8
        ) as outpool:
            l_sb = inpool.tile([128, HL, W], mybir.dt.bfloat16)
            r_sb = inpool.tile([128, HL, W], mybir.dt.bfloat16)
            nc.sync.dma_start(out=l_sb[:], in_=l.ap()[:, :, :1024])
            nc.scalar.dma_start(out=r_sb[:], in_=r.ap()[:, :, :1024])
            for d in range(MAX_DISP):
                t = outpool.tile([128, HL * W], mybir.dt.float16)
                tv = t[:, : HL * (W - d)].rearrange("p (h w) -> p h w", h=HL)
                # GpSimd co-produces every 3rd disparity so early-stream DVE
                # latency never starves the two DMA rings
                ceng = nc.gpsimd if d % 3 == 2 else nc.vector
                ceng.tensor_sub(tv, l_sb[:, :, d:], r_sb[:, :, : W - d])
                oap = (
                    out.ap()[OFF[d] : OFF[d + 1]]
                    .rearrange("(r c) -> r c", c=OROW[d])[:, : OPAY[d]]
                )
                deng = nc.sync if d % 2 == 0 else nc.scalar
                deng.dma_start(out=oap, in_=t[:, : HL * (W - d)])

    nc.compile()
    _CACHE["nc"] = nc
    return nc


def _pad_rows(x):  # (G, H, W) bf16 -> (G, IROWS, IPAD)
    flat = x.reshape(G, IROWS, 1024)
    padded = np.zeros((G, IROWS, IPAD), x.dtype)
    padded[:, :, :1024] = flat
    return padded


def make_in_maps(l_fmap, r_fmap):
    import ml_dtypes

    bf16 = ml_dtypes.bfloat16
    l_flat = np.asarray(l_fmap, dtype=np.float32).astype(bf16).reshape(N * C, H, W)
    r_flat = np.asarray(r_fmap, dtype=np.float32).astype(bf16).reshape(N * C, H, W)
    return [
        {
            "l": _pad_rows(l_flat[k * G : (k + 1) * G]),
            "r": _pad_rows(r_flat[k * G : (k + 1) * G]),
        }
        for k in range(NCORES)
    ]


def gather(results):
    out = np.empty((N * C, MAX_DISP, H, W), np.float32)
    for k, res in enumerate(results):
        flat = res["out"]  # (OSIZE,) fp16
        dst = out[k * G : (k + 1) * G]
        for d in range(MAX_DISP):
            seg = flat[OFF[d] : OFF[d + 1]].reshape(256, OROW[d])[:, : OPAY[d]]
            dst[:, d, :, :d] = 1.0
            # row (p, r) holds h-rows h_hi*8 + r*4 + [0..4), p = g*16 + h_hi
            dst[:, d, :, d:] = seg.reshape(G, H, W - d)
    return out.reshape(N, C, MAX_DISP, H, W)


def kernel(l_fmap, r_fmap):
    nc = build_bass()
    in_maps = make_in_maps(l_fmap, r_fmap)
    res = run_bass_kernel_spmd(nc, in_maps, core_ids=list(range(NCORES)))
    return gather(res.results)


# revision 10
# speedup vs baseline: 1.5386x; 1.5386x over previous
"""Cost-volume kernel for Trainium2 (Bass/Tile), SPMD over 8 NeuronCores.

out[n, c, d, h, x] = l[n, c, h, x] - r[n, c, h, x - d]  for x >= d, else 1.0
shapes: l, r = (2, 32, 128, 256) f32 -> out = (2, 32, 48, 128, 256) f32

Sharding: the 64 (n, c) pairs split 8 ways -> G=8 channels per core; no
cross-core communication.

The kernel is output-write bound: trace analysis showed all 16 SDMA engines
~100% busy (2 KB descriptors, ~22.4 GB/s per engine, ~360 GB/s aggregate),
so the levers are all byte-count:
  1. fp16 device pipeline (inputs pre-cast on host, DVE subtract fp16,
     output DMA fp16) — halves traffic vs f32; ~5e-4 scale-rel error
     against the 2e-2 gate. Host upcasts on gather.
  2. The constant x < d triangle (9.2% of output) is never written: per
     disparity the DVE writes a packed [128, 8*(W-d)] tile; the DMA lands
     each partition's payload as two DRAM rows of 1024-4d elements (+8
     pad), keeping ~2 KB single-fragment descriptors. The host scatters
     the valid region into the final array and fills the triangle with 1.

Per-core layout: SBUF partition p = (g, h_hi), per-partition free dims
(h_lo=8, w). DRAM payload rows are padded by 8 elements (the 16 B gap
defeats descriptor coalescing), so each row is one descriptor and the
outer DRAM AP dim (256) sprays descriptors across all 16 SDMA engines.
Measured on HW: an outer dim of 8 engages only 8 engines (halves DMA
bandwidth); 8 KB descriptors run at ~0.7x the per-engine rate of 2 KB
ones. One DVE subtract per disparity covers all channels; output DMAs
alternate between the two HWDGE rings.
"""

import numpy as np

import concourse.bacc as bacc
import concourse.mybir as mybir
import concourse.tile as tile
from concourse.bass_utils import run_bass_kernel_spmd

MAX_DISP = 48
N, C, H, W = 2, 32, 128, 256
NCORES = 8
G = (N * C) // NCORES  # 8 (n, c) channels per core
HL = 8  # h_lo rows per partition; 128 partitions = G * (H // HL)
IROWS = H * W // 1024  # 32 input payload rows of 1024 fp16 per g
IPAD = 1032
# output: per disparity, 256 DRAM rows (2 per partition) of 1024-4d payload
# elements (= 4 h-rows of W-d) padded by 8
OROW = [1032 - 4 * d for d in range(MAX_DISP)]
OPAY = [1024 - 4 * d for d in range(MAX_DISP)]
OFF = np.cumsum([0] + [256 * r for r in OROW]).tolist()
OSIZE = OFF[-1]

_CACHE = {}


def build_bass():
    if "nc" in _CACHE:
        return _CACHE["nc"]
    nc = bacc.Bacc("TRN2", target_bir_lowering=False, debug=False)
    l = nc.dram_tensor("l", (G, IROWS, IPAD), mybir.dt.float16, kind="ExternalInput")
    r = nc.dram_tensor("r", (G, IROWS, IPAD), mybir.dt.float16, kind="ExternalInput")
    out = nc.dram_tensor("out", (OSIZE,), mybir.dt.float16, kind="ExternalOutput")

    with tile.TileContext(nc) as tc:
        with tc.tile_pool(name="inp", bufs=1) as inpool, tc.tile_pool(
            name="outp", bufs=8
        ) as outpool:
            l_sb = inpool.tile([128, HL, W], mybir.dt.float16)
            r_sb = inpool.tile([128, HL, W], mybir.dt.float16)
            nc.sync.dma_start(out=l_sb[:], in_=l.ap()[:, :, :1024])
            nc.scalar.dma_start(out=r_sb[:], in_=r.ap()[:, :, :1024])
            for d in range(MAX_DISP):
                t = outpool.tile([128, HL * W], mybir.dt.float16)
                tv = t[:, : HL * (W - d)].rearrange("p (h w) -> p h w", h=HL)
                nc.vector.tensor_sub(tv, l_sb[:, :, d:], r_sb[:, :, : W - d])
                oap = (
                    out.ap()[OFF[d] : OFF[d + 1]]
                    .rearrange("(r c) -> r c", c=OROW[d])[:, : OPAY[d]]
                )
                # both HWDGE rings stream each tile concurrently (one
                # partition-half each): halves the pipeline latency quantum
                # so DVE production never starves a ring during ramp-up
                nc.sync.dma_start(
                    out=oap[0:128], in_=t[0:64, : HL * (W - d)]
                )
                nc.scalar.dma_start(
                    out=oap[128:256], in_=t[64:128, : HL * (W - d)]
                )

    nc.compile()
    _CACHE["nc"] = nc
    return nc


def _pad_rows(x):  # (G, H, W) fp16 -> (G, IROWS, IPAD)
    flat = x.reshape(G, IROWS, 1024)
    padded = np.zeros((G, IROWS, IPAD), x.dtype)
    padded[:, :, :1024] = flat
    return padded


def make_in_maps(l_fmap, r_fmap):
    l_flat = np.asarray(l_fmap, dtype=np.float16).reshape(N * C, H, W)
    r_flat = np.asarray(r_fmap, dtype=np.float16).reshape(N * C, H, W)
    return [
        {
            "l": _pad_rows(l_flat[k * G : (k + 1) * G]),
            "r": _pad_rows(r_flat[k * G : (k + 1) * G]),
        }
        for k in range(NCORES)
    ]


def gather(results):
    out = np.empty((N * C, MAX_DISP, H, W), np.float32)
    for k, res in enumerate(results):
        flat = res["out"]  # (OSIZE,) fp16
        dst = out[k * G : (k + 1) * G]
        for d in range(MAX_DISP):
            seg = flat[OFF[d] : OFF[d + 1]].reshape(256, OROW[d])[:, : OPAY[d]]
            dst[:, d, :, :d] = 1.0
            # row (p, r) holds h-rows h_hi*8 + r*4 + [0..4), p = g*16 + h_hi
            dst[:, d, :, d:] = seg.reshape(G, H, W - d)
    return out.reshape(N, C, MAX_DISP, H, W)


def kernel(l_fmap, r_fmap):
    nc = build_bass()
    in_maps = make_in_maps(l_fmap, r_fmap)
    res = run_bass_kernel_spmd(nc, in_maps, core_ids=list(range(NCORES)))
    return gather(res.results)


# revision 11
# speedup vs baseline: 1.5511x; 1.0081x over previous
"""Cost-volume kernel for Trainium2 (Bass/Tile), SPMD over 8 NeuronCores.

out[n, c, d, h, x] = l[n, c, h, x] - r[n, c, h, x - d]  for x >= d, else 1.0
shapes: l, r = (2, 32, 128, 256) f32 -> out = (2, 32, 48, 128, 256) f32

Sharding: the 64 (n, c) pairs split 8 ways -> G=8 channels per core; no
cross-core communication.

The kernel is output-write bound: trace analysis showed all 16 SDMA engines
~100% busy (2 KB descriptors, ~22.4 GB/s per engine, ~360 GB/s aggregate),
so the levers are all byte-count:
  1. fp16 device pipeline (inputs pre-cast on host, DVE subtract fp16,
     output DMA fp16) — halves traffic vs f32; ~5e-4 scale-rel error
     against the 2e-2 gate. Host upcasts on gather.
  2. The constant x < d triangle (9.2% of output) is never written: per
     disparity the DVE writes a packed [128, 8*(W-d)] tile; the DMA lands
     each partition's payload as two DRAM rows of 1024-4d elements (+8
     pad), keeping ~2 KB single-fragment descriptors. The host scatters
     the valid region into the final array and fills the triangle with 1.

Per-core layout: SBUF partition p = (g, h_hi), per-partition free dims
(h_lo=8, w). DRAM payload rows are padded by 8 elements (the 16 B gap
defeats descriptor coalescing), so each row is one descriptor and the
outer DRAM AP dim (256) sprays descriptors across all 16 SDMA engines.
Measured on HW: an outer dim of 8 engages only 8 engines (halves DMA
bandwidth); 8 KB descriptors run at ~0.7x the per-engine rate of 2 KB
ones. One DVE subtract per disparity covers all channels; output DMAs
alternate between the two HWDGE rings.
"""

import numpy as np

import concourse.bacc as bacc
import concourse.mybir as mybir
import concourse.tile as tile
from concourse.bass_utils import run_bass_kernel_spmd

MAX_DISP = 48
N, C, H, W = 2, 32, 128, 256
NCORES = 8
G = (N * C) // NCORES  # 8 (n, c) channels per core
HL = 8  # h_lo rows per partition; 128 partitions = G * (H // HL)
IROWS = H * W // 1024  # 32 input payload rows of 1024 fp16 per g
IPAD = 1032
# output: per disparity, 256 DRAM rows (2 per partition) of 1024-4d payload
# elements (= 4 h-rows of W-d) padded by 8
OROW = [1032 - 4 * d for d in range(MAX_DISP)]
OPAY = [1024 - 4 * d for d in range(MAX_DISP)]
OFF = np.cumsum([0] + [256 * r for r in OROW]).tolist()
OSIZE = OFF[-1]

_CACHE = {}


def build_bass():
    if "nc" in _CACHE:
        return _CACHE["nc"]
    nc = bacc.Bacc("TRN2", target_bir_lowering=False, debug=False)
    l = nc.dram_tensor("l", (G, IROWS, IPAD), mybir.dt.float16, kind="ExternalInput")
    r = nc.dram_tensor("r", (G, IROWS, IPAD), mybir.dt.float16, kind="ExternalInput")
    out = nc.dram_tensor("out", (OSIZE,), mybir.dt.float16, kind="ExternalOutput")

    with tile.TileContext(nc) as tc:
        with tc.tile_pool(name="inp", bufs=1) as inpool, tc.tile_pool(
            name="outp", bufs=8
        ) as outpool:
            l_sb = inpool.tile([128, HL, W], mybir.dt.float16)
            r_sb = inpool.tile([128, HL, W], mybir.dt.float16)
            nc.sync.dma_start(out=l_sb[:], in_=l.ap()[:, :, :1024])
            nc.scalar.dma_start(out=r_sb[:], in_=r.ap()[:, :, :1024])
            for d in range(MAX_DISP):
                t = outpool.tile([128, HL * W], mybir.dt.float16)
                tv = t[:, : HL * (W - d)].rearrange("p (h w) -> p h w", h=HL)
                nc.vector.tensor_sub(tv, l_sb[:, :, d:], r_sb[:, :, : W - d])
                oap = (
                    out.ap()[OFF[d] : OFF[d + 1]]
                    .rearrange("(r c) -> r c", c=OROW[d])[:, : OPAY[d]]
                )
                if d < 8:
                    # during pipeline ramp-up both HWDGE rings stream each
                    # tile concurrently (one partition-half each) so DVE
                    # latency jitter never starves a ring; steady-state uses
                    # whole-tile transfers (fewer per-transfer boundaries)
                    nc.sync.dma_start(out=oap[0:128], in_=t[0:64, : HL * (W - d)])
                    nc.scalar.dma_start(
                        out=oap[128:256], in_=t[64:128, : HL * (W - d)]
                    )
                else:
                    deng = nc.sync if d % 2 == 0 else nc.scalar
                    deng.dma_start(out=oap, in_=t[:, : HL * (W - d)])

    nc.compile()
    _CACHE["nc"] = nc
    return nc


def _pad_rows(x):  # (G, H, W) fp16 -> (G, IROWS, IPAD)
    flat = x.reshape(G, IROWS, 1024)
    padded = np.zeros((G, IROWS, IPAD), x.dtype)
    padded[:, :, :1024] = flat
    return padded


def make_in_maps(l_fmap, r_fmap):
    l_flat = np.asarray(l_fmap, dtype=np.float16).reshape(N * C, H, W)
    r_flat = np.asarray(r_fmap, dtype=np.float16).reshape(N * C, H, W)
    return [
        {
            "l": _pad_rows(l_flat[k * G : (k + 1) * G]),
            "r": _pad_rows(r_flat[k * G : (k + 1) * G]),
        }
        for k in range(NCORES)
    ]


def gather(results):
    out = np.empty((N * C, MAX_DISP, H, W), np.float32)
    for k, res in enumerate(results):
        flat = res["out"]  # (OSIZE,) fp16
        dst = out[k * G : (k + 1) * G]
        for d in range(MAX_DISP):
            seg = flat[OFF[d] : OFF[d + 1]].reshape(256, OROW[d])[:, : OPAY[d]]
            dst[:, d, :, :d] = 1.0
            # row (p, r) holds h-rows h_hi*8 + r*4 + [0..4), p = g*16 + h_hi
            dst[:, d, :, d:] = seg.reshape(G, H, W - d)
    return out.reshape(N, C, MAX_DISP, H, W)


def kernel(l_fmap, r_fmap):
    nc = build_bass()
    in_maps = make_in_maps(l_fmap, r_fmap)
    res = run_bass_kernel_spmd(nc, in_maps, core_ids=list(range(NCORES)))
    return gather(res.results)


# revision 12
# speedup vs baseline: 1.6465x; 1.0615x over previous
"""Cost-volume kernel for Trainium2 (Bass/Tile), SPMD over 8 NeuronCores.

out[n, c, d, h, x] = l[n, c, h, x] - r[n, c, h, x - d]  for x >= d, else 1.0
shapes: l, r = (2, 32, 128, 256) f32 -> out = (2, 32, 48, 128, 256) f32

Sharding: the 64 (n, c) pairs split 8 ways -> G=8 channels per core; no
cross-core communication.

The kernel is output-write bound: trace analysis showed all 16 SDMA engines
~100% busy (2 KB descriptors, ~22.4 GB/s per engine, ~360 GB/s aggregate),
so the levers are all byte-count:
  1. fp16 device pipeline (inputs pre-cast on host, DVE subtract fp16,
     output DMA fp16) — halves traffic vs f32; ~5e-4 scale-rel error
     against the 2e-2 gate. Host upcasts on gather.
  2. The constant x < d triangle (9.2% of output) is never written: per
     disparity the DVE writes a packed [128, 8*(W-d)] tile; the DMA lands
     each partition's payload as two DRAM rows of 1024-4d elements (+8
     pad), keeping ~2 KB single-fragment descriptors. The host scatters
     the valid region into the final array and fills the triangle with 1.

Per-core layout: SBUF partition p = (g, h_hi), per-partition free dims
(h_lo=8, w). DRAM payload rows are padded by 8 elements (the 16 B gap
defeats descriptor coalescing), so each row is one descriptor and the
outer DRAM AP dim (256) sprays descriptors across all 16 SDMA engines.
Measured on HW: an outer dim of 8 engages only 8 engines (halves DMA
bandwidth); 8 KB descriptors run at ~0.7x the per-engine rate of 2 KB
ones. One DVE subtract per disparity covers all channels; output DMAs
alternate between the two HWDGE rings.
"""

import numpy as np

import concourse.bacc as bacc
import concourse.mybir as mybir
import concourse.tile as tile
from concourse.bass_utils import run_bass_kernel_spmd

MAX_DISP = 48
N, C, H, W = 2, 32, 128, 256
NCORES = 8
G = (N * C) // NCORES  # 8 (n, c) channels per core
HL = 8  # h_lo rows per partition; 128 partitions = G * (H // HL)
IROWS = H * W // 1024  # 32 input payload rows of 1024 fp16 per g
IPAD = 1032
# output: per disparity, 256 DRAM rows (2 per partition) of 1024-4d payload
# elements (= 4 h-rows of W-d) padded by 8
OROW = [1032 - 4 * d for d in range(MAX_DISP)]
OPAY = [1024 - 4 * d for d in range(MAX_DISP)]
OFF = np.cumsum([0] + [256 * r for r in OROW]).tolist()
OSIZE = OFF[-1]

_CACHE = {}


def build_bass():
    if "nc" in _CACHE:
        return _CACHE["nc"]
    nc = bacc.Bacc("TRN2", target_bir_lowering=False, debug=False)
    l = nc.dram_tensor("l", (G, IROWS, IPAD), mybir.dt.float16, kind="ExternalInput")
    r = nc.dram_tensor("r", (G, IROWS, IPAD), mybir.dt.float16, kind="ExternalInput")
    out = nc.dram_tensor("out", (OSIZE,), mybir.dt.float16, kind="ExternalOutput")

    with tile.TileContext(nc) as tc:
        with tc.tile_pool(name="inp", bufs=1) as inpool, tc.tile_pool(
            name="outp", bufs=8
        ) as outpool:
            l_sb = inpool.tile([128, HL, W], mybir.dt.float16)
            r_sb = inpool.tile([128, HL, W], mybir.dt.float16)
            nc.sync.dma_start(out=l_sb[:], in_=l.ap()[:, :, :1024])
            nc.scalar.dma_start(out=r_sb[:], in_=r.ap()[:, :, :1024])
            for d in range(MAX_DISP):
                t = outpool.tile([128, HL * W], mybir.dt.float16)
                tv = t[:, : HL * (W - d)].rearrange("p (h w) -> p h w", h=HL)
                nc.vector.tensor_sub(tv, l_sb[:, :, d:], r_sb[:, :, : W - d])
                oap = (
                    out.ap()[OFF[d] : OFF[d + 1]]
                    .rearrange("(r c) -> r c", c=OROW[d])[:, : OPAY[d]]
                )
                deng = nc.sync if d % 2 == 0 else nc.scalar
                deng.dma_start(out=oap, in_=t[:, : HL * (W - d)])

    nc.compile()
    _CACHE["nc"] = nc
    return nc


def _pad_rows(x):  # (G, H, W) fp16 -> (G, IROWS, IPAD)
    flat = x.reshape(G, IROWS, 1024)
    padded = np.zeros((G, IROWS, IPAD), x.dtype)
    padded[:, :, :1024] = flat
    return padded


def make_in_maps(l_fmap, r_fmap):
    l_flat = np.asarray(l_fmap, dtype=np.float16).reshape(N * C, H, W)
    r_flat = np.asarray(r_fmap, dtype=np.float16).reshape(N * C, H, W)
    return [
        {
            "l": _pad_rows(l_flat[k * G : (k + 1) * G]),
            "r": _pad_rows(r_flat[k * G : (k + 1) * G]),
        }
        for k in range(NCORES)
    ]


def gather(results):
    out = np.empty((N * C, MAX_DISP, H, W), np.float32)
    for k, res in enumerate(results):
        flat = res["out"]  # (OSIZE,) fp16
        dst = out[k * G : (k + 1) * G]
        for d in range(MAX_DISP):
            seg = flat[OFF[d] : OFF[d + 1]].reshape(256, OROW[d])[:, : OPAY[d]]
            dst[:, d, :, :d] = 1.0
            # row (p, r) holds h-rows h_hi*8 + r*4 + [0..4), p = g*16 + h_hi
            dst[:, d, :, d:] = seg.reshape(G, H, W - d)
    return out.reshape(N, C, MAX_DISP, H, W)


def kernel(l_fmap, r_fmap):
    nc = build_bass()
    in_maps = make_in_maps(l_fmap, r_fmap)
    res = run_bass_kernel_spmd(nc, in_maps, core_ids=list(range(NCORES)))
    return gather(res.results)


# revision 13
# speedup vs baseline: 1.7681x; 1.0739x over previous
"""Cost-volume kernel for Trainium2 (Bass/Tile), SPMD over 8 NeuronCores.

out[n, c, d, h, x] = l[n, c, h, x] - r[n, c, h, x - d]  for x >= d, else 1.0
shapes: l, r = (2, 32, 128, 256) f32 -> out = (2, 32, 48, 128, 256) f32

Sharding: the 64 (n, c) pairs split 8 ways -> G=8 channels per core; no
cross-core communication.

The kernel is output-write bound: trace analysis showed all 16 SDMA engines
~100% busy (2 KB descriptors, ~22.4 GB/s per engine, ~360 GB/s aggregate),
so the levers are all byte-count:
  1. fp16 device pipeline (inputs pre-cast on host, DVE subtract fp16,
     output DMA fp16) — halves traffic vs f32; ~5e-4 scale-rel error
     against the 2e-2 gate. Host upcasts on gather.
  2. The constant x < d triangle (9.2% of output) is never written: per
     disparity the DVE writes a packed [128, 8*(W-d)] tile; the DMA lands
     each partition's payload as two DRAM rows of 1024-4d elements (+8
     pad), keeping ~2 KB single-fragment descriptors. The host scatters
     the valid region into the final array and fills the triangle with 1.

Per-core layout: SBUF partition p = (g, h_hi), per-partition free dims
(h_lo=8, w). DRAM payload rows are padded by 8 elements (the 16 B gap
defeats descriptor coalescing), so each row is one descriptor and the
outer DRAM AP dim (256) sprays descriptors across all 16 SDMA engines.
Measured on HW: an outer dim of 8 engages only 8 engines (halves DMA
bandwidth); 8 KB descriptors run at ~0.7x the per-engine rate of 2 KB
ones. One DVE subtract per disparity covers all channels; output DMAs
alternate between the two HWDGE rings.
"""

import numpy as np

import concourse.bacc as bacc
import concourse.mybir as mybir
import concourse.tile as tile
from concourse.bass_utils import run_bass_kernel_spmd

MAX_DISP = 48
N, C, H, W = 2, 32, 128, 256
NCORES = 8
G = (N * C) // NCORES  # 8 (n, c) channels per core
HL = 8  # h_lo rows per partition; 128 partitions = G * (H // HL)
IROWS = H * W // 1024  # 32 input payload rows of 1024 fp16 per g
IPAD = 1032
# output: per disparity, 128 DRAM rows (1 per partition) of 2048-8d payload
# elements (= 8 h-rows of W-d) padded by 8 -> ~4 KB descriptors
OROW = [2056 - 8 * d for d in range(MAX_DISP)]
OPAY = [2048 - 8 * d for d in range(MAX_DISP)]
OFF = np.cumsum([0] + [128 * r for r in OROW]).tolist()
OSIZE = OFF[-1]

_CACHE = {}


def build_bass():
    if "nc" in _CACHE:
        return _CACHE["nc"]
    nc = bacc.Bacc("TRN2", target_bir_lowering=False, debug=False)
    l = nc.dram_tensor("l", (G, IROWS, IPAD), mybir.dt.float16, kind="ExternalInput")
    r = nc.dram_tensor("r", (G, IROWS, IPAD), mybir.dt.float16, kind="ExternalInput")
    out = nc.dram_tensor("out", (OSIZE,), mybir.dt.float16, kind="ExternalOutput")

    with tile.TileContext(nc) as tc:
        with tc.tile_pool(name="inp", bufs=1) as inpool, tc.tile_pool(
            name="outp", bufs=8
        ) as outpool:
            l_sb = inpool.tile([128, HL, W], mybir.dt.float16)
            r_sb = inpool.tile([128, HL, W], mybir.dt.float16)
            nc.sync.dma_start(out=l_sb[:], in_=l.ap()[:, :, :1024])
            nc.scalar.dma_start(out=r_sb[:], in_=r.ap()[:, :, :1024])
            for d in range(MAX_DISP):
                t = outpool.tile([128, HL * W], mybir.dt.float16)
                tv = t[:, : HL * (W - d)].rearrange("p (h w) -> p h w", h=HL)
                nc.vector.tensor_sub(tv, l_sb[:, :, d:], r_sb[:, :, : W - d])
                oap = (
                    out.ap()[OFF[d] : OFF[d + 1]]
                    .rearrange("(r c) -> r c", c=OROW[d])[:, : OPAY[d]]
                )
                deng = nc.sync if d % 2 == 0 else nc.scalar
                deng.dma_start(out=oap, in_=t[:, : HL * (W - d)])

    nc.compile()
    _CACHE["nc"] = nc
    return nc


def _pad_rows(x):  # (G, H, W) fp16 -> (G, IROWS, IPAD)
    flat = x.reshape(G, IROWS, 1024)
    padded = np.zeros((G, IROWS, IPAD), x.dtype)
    padded[:, :, :1024] = flat
    return padded


def make_in_maps(l_fmap, r_fmap):
    l_flat = np.asarray(l_fmap, dtype=np.float16).reshape(N * C, H, W)
    r_flat = np.asarray(r_fmap, dtype=np.float16).reshape(N * C, H, W)
    return [
        {
            "l": _pad_rows(l_flat[k * G : (k + 1) * G]),
            "r": _pad_rows(r_flat[k * G : (k + 1) * G]),
        }
        for k in range(NCORES)
    ]


def gather(results):
    out = np.empty((N * C, MAX_DISP, H, W), np.float32)
    for k, res in enumerate(results):
        flat = res["out"]  # (OSIZE,) fp16
        dst = out[k * G : (k + 1) * G]
        for d in range(MAX_DISP):
            seg = flat[OFF[d] : OFF[d + 1]].reshape(128, OROW[d])[:, : OPAY[d]]
            dst[:, d, :, :d] = 1.0
            # row (p, r) holds h-rows h_hi*8 + r*4 + [0..4), p = g*16 + h_hi
            dst[:, d, :, d:] = seg.reshape(G, H, W - d)
    return out.reshape(N, C, MAX_DISP, H, W)


def kernel(l_fmap, r_fmap):
    nc = build_bass()
    in_maps = make_in_maps(l_fmap, r_fmap)
    res = run_bass_kernel_spmd(nc, in_maps, core_ids=list(range(NCORES)))
    return gather(res.results)
